# revision 25
# baseline (speedup 1.0000x reference)
"""Trainium2 Bass kernel for nn_ContextEncoderLayer (per-position cross-attention
encoder layer).  Shards the sequence dim L across 8 NeuronCores; each core runs an
identical Bass/Tile program on its 256-position shard.

v2 layout strategy (per core, L_C=256 positions, 64 row-tiles of 128 (l,d)-rows):
  The K and V projections (2x 8.6 GMAC each) are eliminated algebraically:
    scores[l,h,d] = X[(l,d),:] . P[l,h,:]   with P = q projected through Wk
                                            (P costs 0.27 GMAC instead of 8.6)
    ctx[l,h,:]    = (sum_d exp[l,d,h] X[(l,d),:]) @ Wv_h / den[l,h]
  Per tile: 8 matmuls xt_c^T @ P~ -> scores PSUM [128,(pos,h)]; ACT exp (4 x 32-row
  slices, mask as bias); exp scattered into a block-diagonal E [128,64]; 8 matmuls
  xn_c^T? no: lhsT=xn chunks, rhs=E -> Y^T [dm,(pos,h)]; per l-tile 128 matmuls
  Yt^T @ Wv_h accumulate ctx [128 pos, 1024].  Softmax den via 0/1-selector
  matmuls into PSUM (no cross-partition reductions).  LN + FFN as before (W1/W2
  streamed, GELU fused into PSUM eviction).
"""

import sys

sys.path.insert(0, "/opt/trn_rl_repo")

from contextlib import ExitStack

import numpy as np
import ml_dtypes

import concourse.bacc as bacc
import concourse.tile as tile
from concourse import mybir
from concourse.bass_utils import run_bass_kernel_spmd
from concourse.masks import make_identity

L, D, DM, H, FF = 2048, 32, 1024, 16, 4096
DH = DM // H  # 64
SCALE = float(np.sqrt(DH))  # 8.0
NCORES = 8
LC = L // NCORES  # 256 positions per core
NT = LC * D // 128  # 64 (l,d)-row tiles per core
NLT = LC // 128  # 2 l-tiles per core
NC_DM = DM // 128  # 8 dm chunks
CH = NC_DM + 1  # 9: 8 chunks + ones-row (bias fold, Q proj only)
NFF = FF // 128  # 32 ff chunks
TPB = NT // NLT  # 32 (l,d)-tiles per l-tile
BF = mybir.dt.bfloat16
F8 = mybir.dt.float8e4
F32 = mybir.dt.float32

_CACHE = {}
import os
DBG_BASE64 = os.environ.get('DBG_BASE64', '1') == '1'
DBG_STRIDED = os.environ.get('DBG_STRIDED', '1') == '1'


def _sel_matrices():
    """G_j [128, 32] for j in 0..7: G_j[p, c] = 1 iff c == 4j + p//32."""
    g = np.zeros((128, 8 * 32), dtype=np.float32)
    for j in range(8):
        for p in range(128):
            g[p, j * 32 + (4 * j + p // 32)] = 1.0
    return g.astype(ml_dtypes.bfloat16)


def _build_nc(repeat=1):
    nc = bacc.Bacc("TRN2", target_bir_lowering=False, debug=False, num_devices=NCORES)

    # ---------------- I/O ----------------
    xt_in = nc.dram_tensor("xt", [NT // 2, 128, 2 * NC_DM * 128], F8, kind="ExternalInput")
    xn_in = nc.dram_tensor("xn", [NT // 2, 128, 2 * DM], BF, kind="ExternalInput")
    wq_in = nc.dram_tensor("wq", [128, CH * 1024], BF, kind="ExternalInput")
    st_in = nc.dram_tensor("st", [128, CH * LC], BF, kind="ExternalInput")
    wkt_in = nc.dram_tensor("wkt", [128, H * NC_DM * 128], BF, kind="ExternalInput")
    wv_in = nc.dram_tensor("wv", [128, NC_DM * 1024], BF, kind="ExternalInput")
    sbv_in = nc.dram_tensor("sbv", [LC, DM], F32, kind="ExternalInput")
    mask_in = nc.dram_tensor("maskp", [128, NT], F32, kind="ExternalInput")
    b1_in = nc.dram_tensor("b1p", [128, NFF], F32, kind="ExternalInput")
    b2_in = nc.dram_tensor("b2p", [1, DM], F32, kind="ExternalInput")
    g1_in = nc.dram_tensor("g1p", [1, DM], F32, kind="ExternalInput")
    be1_in = nc.dram_tensor("be1p", [1, DM], F32, kind="ExternalInput")
    g2_in = nc.dram_tensor("g2p", [1, DM], F32, kind="ExternalInput")
    be2_in = nc.dram_tensor("be2p", [1, DM], F32, kind="ExternalInput")
    w1_in = nc.dram_tensor("w1p", [NFF, 128, 1024], BF, kind="ExternalInput")
    w2_in = nc.dram_tensor("w2p", [NFF, 128, 1024], BF, kind="ExternalInput")
    out = nc.dram_tensor("out", [LC, DM], F32, kind="ExternalOutput")

    g_const = nc.inline_tensor(np.asarray(_sel_matrices()), name="gsel")

    AL = mybir.AluOpType
    AF = mybir.ActivationFunctionType

    with tile.TileContext(nc) as tc, ExitStack() as top:
        consts = top.enter_context(tc.tile_pool(name="consts", bufs=1))
        work = top.enter_context(tc.tile_pool(name="work", bufs=1))

        # ------- constants / params resident in SBUF -------
        wq_sb = consts.tile([128, CH * 1024], BF)
        nc.sync.dma_start(wq_sb[:], wq_in[:])
        st_sb = consts.tile([128, CH * LC], BF)
        nc.sync.dma_start(st_sb[:], st_in[:])
        wkt_sb = consts.tile([128, H * NC_DM * 128], BF)
        nc.sync.dma_start(wkt_sb[:], wkt_in[:])
        wv_sb = consts.tile([128, NC_DM * 1024], BF)
        nc.sync.dma_start(wv_sb[:], wv_in[:])
        mask_sb = consts.tile([128, NT], F32)
        nc.sync.dma_start(mask_sb[:], mask_in[:])
        b1_sb = consts.tile([128, NFF], F32)
        nc.sync.dma_start(b1_sb[:], b1_in[:])
        g_sb = consts.tile([128, 8 * 32], BF)
        nc.sync.dma_start(g_sb[:], g_const[:])
        ident = consts.tile([128, 128], BF)
        make_identity(nc, ident[:])
        eps_sb = consts.tile([128, 1], F32)
        nc.vector.memset(eps_sb[:], 1e-5)
        # block-diagonal exp matrices (double-buffered manually; off-diagonal
        # stays zero forever, diag blocks rewritten per tile)
        e_bufs = [consts.tile([128, 64], BF, name=f"ebuf{i}") for i in range(2)]
        nc.vector.memset(e_bufs[0][:], 0.0)
        nc.vector.memset(e_bufs[1][:], 0.0)

        def rep128(name, src):  # [1, DM] -> [128, DM] partition-broadcast (bf16)
            t = consts.tile([128, DM], BF, name=name)
            nc.gpsimd.dma_start(t[:], src[0:1, :].broadcast_to([128, DM]))
            return t

        g1_rep = rep128("g1_rep", g1_in)
        be1_rep = rep128("be1_rep", be1_in)

        xres = top.enter_context(tc.tile_pool(name="xres", bufs=1))
        xtp = top.enter_context(tc.tile_pool(name="xtp", bufs=1))
        qtp = top.enter_context(tc.tile_pool(name="qtp", bufs=1))
        ptp = top.enter_context(tc.tile_pool(name="ptp", bufs=1))
        ytp = top.enter_context(tc.tile_pool(name="ytp", bufs=1))

        for _rep in range(repeat):
            x_tiles = []
            xT_sb = xtp.tile([128, NC_DM * LC], BF, name=f"xT{_rep}", tag="xT")
            qT_sb = qtp.tile([128, NC_DM * LC], BF, name=f"qT{_rep}", tag="qT")

            # ---------------- phase Q: q = (src @ Wq + bq)/8, transposed ----------------
            with tc.tile_pool(name="qps", bufs=2, space="PSUM") as qpsp, \
                 tc.tile_pool(name="qtps", bufs=2, space="PSUM") as qtpsp:
                for lt in range(NLT):
                    qps = qpsp.tile([128, 1024], F32, name=f"qps{_rep}_{lt}", tag="qps")
                    for h2 in range(2):
                        for c in range(CH):
                            if c < NC_DM:
                                lhsT = st_sb[:, c * LC + lt * 128 : c * LC + (lt + 1) * 128]
                                rhs = wq_sb[:, c * 1024 + h2 * 512 : c * 1024 + h2 * 512 + 512]
                            else:
                                lhsT = st_sb[0:1, c * LC + lt * 128 : c * LC + (lt + 1) * 128]
                                rhs = wq_sb[0:1, c * 1024 + h2 * 512 : c * 1024 + h2 * 512 + 512]
                            nc.tensor.matmul(
                                qps[:, h2 * 512 : (h2 + 1) * 512],
                                lhsT,
                                rhs,
                                start=(c == 0),
                                stop=(c == CH - 1),
                            )
                    q_bf = work.tile([128, 1024], BF, name=f"q_bf{_rep}_{lt}", tag="qbf")
                    nc.scalar.copy(q_bf[:], qps[:])
                    for cq in range(NC_DM):
                        tp = qtpsp.tile([128, 128], BF, name=f"{_rep}_qtp{lt}_{cq}", tag="qtp")
                        nc.tensor.transpose(tp[:], q_bf[:, cq * 128 : (cq + 1) * 128], ident[:])
                        nc.scalar.copy(
                            qT_sb[:, cq * LC + lt * 128 : cq * LC + (lt + 1) * 128], tp[:]
                        )

            # ---------------- attention (per l-tile) ----------------
            with ExitStack() as pb:
                sc_psp = pb.enter_context(tc.tile_pool(name="sc_ps", bufs=2, space="PSUM"))
                yt_psp = pb.enter_context(tc.tile_pool(name="yt_ps", bufs=2, space="PSUM"))
                den_psp = pb.enter_context(tc.tile_pool(name="den_ps", bufs=1, space="PSUM"))
                tp_psp = pb.enter_context(tc.tile_pool(name="tp_ps", bufs=1, space="PSUM"))
                ctx_psp = pb.enter_context(tc.tile_pool(name="ctx_ps", bufs=1, space="PSUM"))
                xt_pool = pb.enter_context(tc.tile_pool(name="xt_pool", bufs=2))
                xn_pool = pb.enter_context(tc.tile_pool(name="xn_pool", bufs=2))
                ex_pool = pb.enter_context(tc.tile_pool(name="ex_pool", bufs=2))
                ln_pool = pb.enter_context(tc.tile_pool(name="ln_pool", bufs=1))
                sm_pool = pb.enter_context(tc.tile_pool(name="sm_pool", bufs=2))

                for lt in range(NLT):
                    # ---- phase P: P~^T[dm, (l,h)] for this l-tile's 128 positions ----
                    # pt layout: [128, c(8) x t(32) x pos(4) x h(16)] -> scores rhs
                    # slice [c*2048 + tt*64 : +64] is contiguous (pos,h)-ordered.
                    pt_sb = ptp.tile([128, NC_DM * 2048], F8, name=f"pt{_rep}_{lt}", tag="pt")
                    pt_v = pt_sb.rearrange(
                        "p (c t pos h) -> p c t pos h", c=8, t=32, pos=4, h=16
                    )
                    for c in range(NC_DM):
                        for hg in range(4):
                            pps = sc_psp.tile(
                                [128, 512], F32, name=f"{_rep}_pps{lt}_{c}_{hg}", tag="sc"
                            )
                            for h4 in range(4):
                                h = hg * 4 + h4
                                lhsT = wkt_sb[:, (h * NC_DM + c) * 128 : (h * NC_DM + c + 1) * 128]
                                rhs = qT_sb[:, (h // 2) * LC + lt * 128 : (h // 2) * LC + (lt + 1) * 128]
                                nc.tensor.matmul(
                                    pps[:, h4 * 128 : (h4 + 1) * 128],
                                    lhsT,
                                    rhs,
                                    start=True,
                                    stop=True,
                                )
                            # strided evict: src [p, h4, t, pos] -> dst [p, c, t, pos, hg*4+h4]
                            if DBG_STRIDED:
                                src = pps.rearrange("p (h4 t pos) -> p h4 t pos", h4=4, t=32, pos=4)
                                dst = pt_v[:, c, :, :, hg * 4 : hg * 4 + 4].rearrange(
                                    "p t pos h4 -> p h4 t pos"
                                )
                            else:
                                src = pps[:]
                                dst = pt_sb[:, c * 2048 + hg * 512 : c * 2048 + (hg + 1) * 512]
                            if (c * 4 + hg) % 2 == 0:
                                nc.scalar.copy(dst, src)
                            else:
                                nc.vector.tensor_copy(dst, src)

                    ctx_ps = ctx_psp.tile([128, 1024], F32, name=f"{_rep}_ctx{lt}", tag="ctx")
                    den_ps = den_psp.tile([128, 16], F32, name=f"{_rep}_den{lt}", tag="den")
                    # yt layout: [128, (c,h) x l] -> ctx lhsT slice is contiguous
                    yt_sb = ytp.tile([128, TPB * 512], BF, name=f"yt{_rep}_{lt}", tag="yt")
                    yt_v = yt_sb.rearrange(
                        "p (c h l) -> p c h l", c=8, h=16, l=128
                    )

                    pending = None

                    def emit_pending(p):
                        (tt2, xn_sb2, E2, ex2) = p
                        yt_ps = yt_psp.tile(
                            [128, 512], F32, name=f"{_rep}_ytps{lt}_{tt2}", tag="ytps"
                        )
                        for c in range(NC_DM):
                            nc.tensor.matmul(
                                yt_ps[:, c * 64 : (c + 1) * 64],
                                xn_sb2[:, c * 128 : (c + 1) * 128],
                                E2[:],
                                start=True,
                                stop=True,
                            )
                        j, g = tt2 % 8, tt2 // 8
                        nc.tensor.matmul(
                            den_ps[32 * g : 32 * (g + 1), :],
                            g_sb[:, j * 32 : (j + 1) * 32],
                            ex2[:],
                            start=(j == 0),
                            stop=(j == 7),
                            tile_position=(0, 32 * g),
                            skip_group_check=True,
                        )
                        # strided evict: src [p, c, pos, h] -> dst yt[p, c, h, l=tt*4+pos]
                        if DBG_STRIDED:
                            src = yt_ps.rearrange("p (c pos h) -> p c pos h", c=8, pos=4, h=16)
                            dst = yt_v[:, :, :, tt2 * 4 : tt2 * 4 + 4].rearrange(
                                "p c h pos -> p c pos h"
                            )
                        else:
                            src = yt_ps[:]
                            dst = yt_sb[:, tt2 * 512 : (tt2 + 1) * 512]
                        if tt2 % 2 == 0:
                            nc.scalar.copy(dst, src)
                        else:
                            nc.vector.tensor_copy(dst, src)

                    for tt in range(TPB):
                        t = lt * TPB + tt
                        if tt % 2 == 0:
                            xt2_sb = xt_pool.tile([128, 2048], F8, name=f"{_rep}_xt{t}", tag="xt")
                            nc.sync.dma_start(xt2_sb[:], xt_in[t // 2])
                            xn2_sb = xn_pool.tile([128, 2048], BF, name=f"{_rep}_xn{t}", tag="xn")
                            nc.scalar.dma_start(xn2_sb[:], xn_in[t // 2])
                        xt_sb = xt2_sb[:, (tt % 2) * 1024 : (tt % 2) * 1024 + 1024]
                        xn_sb = xn2_sb[:, (tt % 2) * 1024 : (tt % 2) * 1024 + 1024]

                        sc_ps = sc_psp.tile([128, 512], F32, name=f"{_rep}_sc{t}", tag="sc")
                        for c in range(NC_DM):
                            nc.tensor.matmul(
                                sc_ps[:, 0:64],
                                xt_sb[:, c * 128 : (c + 1) * 128],
                                pt_sb[:, c * 2048 + tt * 64 : c * 2048 + (tt + 1) * 64],
                                start=(c == 0),
                                stop=(c == NC_DM - 1),
                            )
                        if pending is not None:
                            emit_pending(pending)
                            pending = None
                        ex = ex_pool.tile([128, 16], BF, name=f"{_rep}_ex{t}", tag="ex")
                        for g in range(4):
                            nc.scalar.activation(
                                ex[32 * g : 32 * (g + 1), :],
                                sc_ps[32 * g : 32 * (g + 1), 16 * g : 16 * (g + 1)],
                                AF.Exp,
                                bias=mask_sb[32 * g : 32 * (g + 1), t : t + 1],
                                scale=1.0,
                            )
                        E = e_bufs[tt % 2]
                        for g in range(4):
                            nc.vector.tensor_copy(
                                E[32 * g : 32 * (g + 1), 16 * g : 16 * (g + 1)],
                                ex[32 * g : 32 * (g + 1), :],
                            )
                        pending = (tt, xn_sb, E, ex)
                    emit_pending(pending)
                    pending = None

                    # ---- ctx = Yt^T @ Wv_h, accumulated over dm chunks ----
                    for h in range(H):
                        for c in range(NC_DM):
                            lhsT = yt_sb[:, (c * 16 + h) * 128 : (c * 16 + h + 1) * 128]
                            nc.tensor.matmul(
                                ctx_ps[:, h * DH : (h + 1) * DH],
                                lhsT,
                                wv_sb[:, c * 1024 + h * DH : c * 1024 + (h + 1) * DH],
                                start=(c == 0),
                                stop=(c == NC_DM - 1),
                            )

                    # ---- l-tile epilogue: normalize, residual(+bv), LN1, x^T ----
                    rd = sm_pool.tile([128, 16], F32, name=f"{_rep}_rd{lt}", tag="rd")
                    nc.vector.reciprocal(rd[:], den_ps[:])
                    ctxn = ln_pool.tile([128, 1024], F32, name=f"{_rep}_ctxn{lt}", tag="ctxn")
                    nc.vector.tensor_tensor(
                        ctxn.rearrange("p (h x) -> p h x", x=DH),
                        ctx_ps.rearrange("p (h x) -> p h x", x=DH),
                        rd.rearrange("p (h o) -> p h o", o=1).broadcast_to([128, H, DH]),
                        AL.mult,
                    )
                    src_sb = ln_pool.tile([128, 1024], F32, name=f"{_rep}_srcsb{lt}", tag="srcsb")
                    nc.sync.dma_start(src_sb[:], sbv_in[lt * 128 : (lt + 1) * 128, :])
                    r = ln_pool.tile([128, 1024], F32, name=f"{_rep}_r{lt}", tag="r")
                    rsum = sm_pool.tile([128, 1], F32, name=f"{_rep}_rsum{lt}", tag="rsum")
                    nc.vector.tensor_tensor(r[:], ctxn[:], src_sb[:], AL.add)
                    nc.vector.tensor_reduce(
                        rsum[:], r[:], axis=mybir.AxisListType.X, op=AL.add
                    )
                    mean = sm_pool.tile([128, 1], F32, name=f"{_rep}_mean{lt}", tag="mean")
                    nc.vector.tensor_scalar_mul(mean[:], rsum[:], 1.0 / DM)
                    xc = ln_pool.tile([128, 1024], F32, name=f"{_rep}_xc{lt}", tag="xc")
                    nc.vector.tensor_scalar(
                        out=xc[:], in0=r[:], scalar1=mean[:], scalar2=None, op0=AL.subtract
                    )
                    sq = ln_pool.tile([128, 1024], F32, name=f"{_rep}_sq{lt}", tag="ctxn")
                    ssq = sm_pool.tile([128, 1], F32, name=f"{_rep}_ssq{lt}", tag="ssq")
                    nc.scalar.activation(sq[:], xc[:], AF.Square, accum_out=ssq[:])
                    std = sm_pool.tile([128, 1], F32, name=f"{_rep}_std{lt}", tag="std")
                    nc.scalar.activation(
                        std[:], ssq[:], AF.Sqrt, bias=eps_sb[:], scale=1.0 / DM
                    )
                    rstd = sm_pool.tile([128, 1], F32, name=f"{_rep}_rstd{lt}", tag="rstd")
                    nc.vector.reciprocal(rstd[:], std[:])
                    xn_ = ln_pool.tile([128, 1024], F32, name=f"{_rep}_xn{lt}", tag="srcsb")
                    nc.vector.tensor_scalar_mul(xn_[:], xc[:], rstd[:])
                    t1 = ln_pool.tile([128, 1024], F32, name=f"{_rep}_t1_{lt}", tag="r")
                    nc.vector.tensor_tensor(t1[:], xn_[:], g1_rep[:], AL.mult)
                    x = xres.tile([128, 1024], F32, name=f"x{_rep}_{lt}", tag=f"x{lt}")
                    x_tiles.append(x)
                    nc.vector.tensor_tensor(x[:], t1[:], be1_rep[:], AL.add)
                    x_bf = work.tile([128, 1024], BF, name=f"{_rep}_xbf{lt}", tag="qbf")
                    nc.vector.tensor_copy(x_bf[:], x[:])
                    for c in range(NC_DM):
                        tp = tp_psp.tile([128, 128], BF, name=f"{_rep}_tp{lt}_{c}", tag="tp")
                        nc.tensor.transpose(tp[:], x_bf[:, c * 128 : (c + 1) * 128], ident[:])
                        nc.scalar.copy(
                            xT_sb[:, c * LC + lt * 128 : c * LC + (lt + 1) * 128], tp[:]
                        )

            # ---------------- phase C: FFN + LN2 ----------------
            with ExitStack() as pc:
                ff_psp = pc.enter_context(tc.tile_pool(name="ff_ps", bufs=2, space="PSUM"))
                o_psp = pc.enter_context(tc.tile_pool(name="o_ps", bufs=1, space="PSUM"))
                w1_pool = pc.enter_context(tc.tile_pool(name="w1_pool", bufs=3))
                w2_pool = pc.enter_context(tc.tile_pool(name="w2_pool", bufs=2))
                ff1_sb = xtp.tile([128, NFF * LC], BF, name=f"ff1_{_rep}", tag="ff1")
                outps = [
                    o_psp.tile([128, 512], F32, name=f"{_rep}_ops{i}", tag=f"ops{i}")
                    for i in range(4)
                ]
                for cc in range(NFF):
                    w1t = w1_pool.tile([128, 1024], BF, name=f"{_rep}_w1t{cc}", tag="w1t")
                    nc.sync.dma_start(w1t[:], w1_in[cc])
                    ffps = ff_psp.tile([128, LC], F32, name=f"{_rep}_ffps{cc}", tag="ffps")
                    for k in range(NC_DM):
                        nc.tensor.matmul(
                            ffps[:],
                            w1t[:, k * 128 : (k + 1) * 128],
                            xT_sb[:, k * LC : (k + 1) * LC],
                            start=(k == 0),
                            stop=(k == NC_DM - 1),
                        )
                    nc.scalar.activation(
                        ff1_sb[:, cc * LC : (cc + 1) * LC],
                        ffps[:],
                        AF.Gelu,
                        bias=b1_sb[:, cc : cc + 1],
                    )
                    w2t = w2_pool.tile([128, 1024], BF, name=f"{_rep}_w2t{cc}", tag="w2t")
                    nc.scalar.dma_start(w2t[:], w2_in[cc])
                    for lt in range(NLT):
                        for h2 in range(2):
                            nc.tensor.matmul(
                                outps[lt * 2 + h2][:],
                                ff1_sb[:, cc * LC + lt * 128 : cc * LC + (lt + 1) * 128],
                                w2t[:, h2 * 512 : (h2 + 1) * 512],
                                start=(cc == 0),
                                stop=(cc == NFF - 1),
                            )

                ln2_pool = pc.enter_context(tc.tile_pool(name="ln2_pool", bufs=1))
                s2_pool = pc.enter_context(tc.tile_pool(name="s2_pool", bufs=2))
                def rep128c(nm, srcdr, tg):
                    t = ln2_pool.tile([128, DM], BF, name=f"{_rep}_{nm}", tag=tg)
                    nc.gpsimd.dma_start(t[:], srcdr[0:1, :].broadcast_to([128, DM]))
                    return t
                g2_rep = rep128c("g2r", g2_in, "g2r")
                be2_rep = rep128c("be2r", be2_in, "be2r")
                b2_rep = rep128c("b2r", b2_in, "b2r")
                for lt in range(NLT):
                    xb2 = ln2_pool.tile([128, 1024], F32, name=f"{_rep}_xb2_{lt}", tag="xb2")
                    nc.vector.tensor_tensor(xb2[:], x_tiles[lt][:], b2_rep[:], AL.add)
                    r2 = ln2_pool.tile([128, 1024], F32, name=f"{_rep}_r2_{lt}", tag="r2")
                    for h2 in range(2):
                        nc.vector.tensor_tensor(
                            r2[:, h2 * 512 : (h2 + 1) * 512],
                            xb2[:, h2 * 512 : (h2 + 1) * 512],
                            outps[lt * 2 + h2][:],
                            AL.add,
                        )
                    rsum2 = s2_pool.tile([128, 1], F32, name=f"{_rep}_rsum2_{lt}", tag="rsum")
                    nc.vector.tensor_reduce(
                        rsum2[:], r2[:], axis=mybir.AxisListType.X, op=AL.add
                    )
                    mean2 = s2_pool.tile([128, 1], F32, name=f"{_rep}_mean2_{lt}", tag="mean")
                    nc.vector.tensor_scalar_mul(mean2[:], rsum2[:], 1.0 / DM)
                    xc2 = ln2_pool.tile([128, 1024], F32, name=f"{_rep}_xc2_{lt}", tag="xc2")
                    nc.vector.tensor_scalar(
                        out=xc2[:], in0=r2[:], scalar1=mean2[:], scalar2=None, op0=AL.subtract
                    )
                    sq2 = ln2_pool.tile([128, 1024], F32, name=f"{_rep}_sq2_{lt}", tag="xb2")
                    ssq2 = s2_pool.tile([128, 1], F32, name=f"{_rep}_ssq2_{lt}", tag="ssq")
                    nc.scalar.activation(sq2[:], xc2[:], AF.Square, accum_out=ssq2[:])
                    std2 = s2_pool.tile([128, 1], F32, name=f"{_rep}_std2_{lt}", tag="std")
                    nc.scalar.activation(
                        std2[:], ssq2[:], AF.Sqrt, bias=eps_sb[:], scale=1.0 / DM
                    )
                    rstd2 = s2_pool.tile([128, 1], F32, name=f"{_rep}_rstd2_{lt}", tag="rstd")
                    nc.vector.reciprocal(rstd2[:], std2[:])
                    xn2 = ln2_pool.tile([128, 1024], F32, name=f"{_rep}_xn2_{lt}", tag="r2")
                    nc.vector.tensor_scalar_mul(xn2[:], xc2[:], rstd2[:])
                    t2 = ln2_pool.tile([128, 1024], F32, name=f"{_rep}_t2_{lt}", tag="xc2")
                    nc.vector.tensor_tensor(t2[:], xn2[:], g2_rep[:], AL.mult)
                    y = ln2_pool.tile([128, 1024], F32, name=f"{_rep}_y{lt}", tag="y")
                    nc.vector.tensor_tensor(y[:], t2[:], be2_rep[:], AL.add)
                    nc.sync.dma_start(out[lt * 128 : (lt + 1) * 128, :], y[:])

    nc.compile()
    return nc


def _prep_core(src_c, tgt_c, mask_c, W):
    """Host-side layout prep for one core's shard.  Returns the in_map dict."""
    bf = ml_dtypes.bfloat16
    X = np.ascontiguousarray(tgt_c.reshape(LC * D, DM))

    # xt: [NT, 128, 8*128]; [t, p, c*128+m] = X[t*128+m, c*128+p]
    xt = X.reshape(NT, 128, NC_DM, 128).transpose(0, 3, 2, 1)
    # xn: [NT, 128, DM] natural row-major
    xn = X.reshape(NT, 128, DM)

    def wprep(Wm, b, scale=1.0):
        # [128, CH*1024]; [p, c*1024+n] = W'[c*128+p, n], row DM = bias
        Wp = np.zeros((CH * 128, DM), dtype=np.float32)
        Wp[:DM] = Wm * scale
        Wp[DM] = b * scale
        return np.ascontiguousarray(
            Wp.reshape(CH, 128, DM).transpose(1, 0, 2).reshape(128, CH * 1024)
        ).astype(bf)

    # st: [128, CH*LC]; [p, c*LC+f] = src_c[f, c*128+p]; chunk 8 row0 = ones
    st = np.zeros((128, CH, LC), dtype=np.float32)
    st[:, :NC_DM, :] = src_c.reshape(LC, NC_DM, 128).transpose(2, 1, 0)
    st[0, NC_DM, :] = 1.0

    # wkt: [128, 16*8*128]; [(h%2)*64+dh, (h*8+c)*128 + m] = Wk[c*128+m, h*64+dh],
    # other 64 partition rows zero (lets the P matmuls contract K=128 at base 0)
    Wk4 = W["Wk"].reshape(NC_DM, 128, H, DH)  # [c, m, h, dh]
    wkt = np.zeros((128, H, NC_DM, 128), dtype=np.float32)  # [p, h, c, m]
    for h in range(H):
        wkt[(h % 2) * 64 : (h % 2) * 64 + 64, h, :, :] = Wk4[:, :, h, :].transpose(
            2, 0, 1
        )

    # wv: [128, 8*1024]; [p, c*1024+n] = Wv[c*128+p, n]
    wv = W["Wv"].reshape(NC_DM, 128, DM).transpose(1, 0, 2)

    w1p = np.ascontiguousarray(
        W["W1"].reshape(NC_DM, 128, NFF, 128).transpose(2, 1, 0, 3).reshape(NFF, 128, 1024)
    ).astype(bf)
    w2p = np.ascontiguousarray(W["W2"].reshape(NFF, 128, DM)).astype(bf)

    return {
        "xt": np.ascontiguousarray(xt.reshape(NT // 2, 2, 128, NC_DM * 128).transpose(0, 2, 1, 3).reshape(NT // 2, 128, 2 * NC_DM * 128)).astype(ml_dtypes.float8_e4m3),
        "xn": np.ascontiguousarray(np.asarray(xn).reshape(NT // 2, 2, 128, DM).transpose(0, 2, 1, 3).reshape(NT // 2, 128, 2 * DM)).astype(bf),
        "wq": wprep(W["Wq"], W["bq"], scale=1.0 / SCALE),
        "st": np.ascontiguousarray(st.reshape(128, CH * LC)).astype(bf),
        "wkt": np.ascontiguousarray(wkt.reshape(128, H * NC_DM * 128)).astype(bf),
        "wv": np.ascontiguousarray(wv.reshape(128, NC_DM * 1024)).astype(bf),
        "sbv": np.ascontiguousarray(src_c + W["bv"]).astype(np.float32),
        "maskp": np.ascontiguousarray(mask_c.reshape(NT, 128).T).astype(np.float32),
        "b1p": np.ascontiguousarray(W["b1"].reshape(NFF, 128).T).astype(np.float32),
        "b2p": W["b2"].reshape(1, DM).astype(np.float32),
        "g1p": W["g1"].reshape(1, DM).astype(np.float32),
        "be1p": W["beta1"].reshape(1, DM).astype(np.float32),
        "g2p": W["g2"].reshape(1, DM).astype(np.float32),
        "be2p": W["beta2"].reshape(1, DM).astype(np.float32),
        "w1p": w1p,
        "w2p": w2p,
    }


def make_in_maps(**inputs):
    inp = {k: np.asarray(v) for k, v in inputs.items()}
    W = {
        k: inp[k]
        for k in ("Wq", "bq", "Wk", "bk", "Wv", "bv", "W1", "b1", "W2", "b2",
                  "g1", "beta1", "g2", "beta2")
    }
    in_maps = []
    for c in range(NCORES):
        sl = slice(c * LC, (c + 1) * LC)
        in_maps.append(_prep_core(inp["src"][sl], inp["target"][sl], inp["attn_mask"][sl], W))
    return in_maps


def get_nc(repeat=1):
    key = ("nc", repeat)
    if key not in _CACHE:
        _CACHE[key] = _build_nc(repeat)
    return _CACHE[key]


def kernel(**inputs) -> np.ndarray:
    nc = get_nc()
    in_maps = make_in_maps(**inputs)
    res = run_bass_kernel_spmd(nc, in_maps, core_ids=list(range(NCORES)))
    return np.concatenate([res.results[c]["out"] for c in range(NCORES)], axis=0)


if __name__ == "__main__":
    import reference

    inputs = {k: np.asarray(v) for k, v in reference.setup_inputs().items()}
    got = kernel(**inputs)
    exp = np.asarray(reference.reference(**inputs))
    err = np.abs(got - exp).max() / np.abs(exp).max()
    print("Relative error:", err)
